# revision 37
# baseline (speedup 1.0000x reference)
"""Trainium2 Bass kernel for a GPT-style transformer block (no attn out-proj).

Sharding (8 cores): attention is tensor-parallel over heads -- core c handles
batch c//4 and heads [4*(c%4), 4*(c%4)+4) over the full 2048-token causal
sequence. The MLP is token-parallel with a batch-interleaved assignment: core
r handles tokens [256*r, 256*r+256) of BOTH batches.

The feature->token reshard runs as TWO pipelined 8-rank AllToAlls, one
per head-pair group: A2A#0 ships heads {0,1} of each core's group while
the core computes heads {2,3}; the receive-side assembly (y normalize,
residual add, LN2 partial stats) for group 0 overlaps A2A#1. Both
collective calls sit after ALL scatter DMAs in program order so their
bounce-copy descriptors cannot block later compute DMAs on the rings.

Attention softmax normalization is deferred past the collective: each core
ships unnormalized sums (plus the denominator row, via an appended ones
column in V) and the receiving core divides after the exchange.

Matmuls run in bf16 (fp32 PSUM accumulate). exp() is split between the
scalar engine (table exp) and the vector engine (Schraudolph bit-trick
exp, ~3% max rel err, which washes out in softmax normalization).
"""

import numpy as np

B, T, C = 2, 2048, 1024
H, D = 16, 64
HPC = 4            # heads per core
VD = HPC * D       # 256 v features per core
N_CORES = 8
TCH = 512          # tokens per core for the MLP (256 from each batch)
TB = 256           # per-batch token block
EPS = 1e-5
FC = 4 * C
CO = C // 128      # 8
FO = FC // 128     # 32
NT = T // 512      # 4 query chunks
TT = T // 128      # 16 token tiles
VS = 128           # padded per-head v slot (64 v + 1 ones + 63 zero)
SLOT = 2 * (D + 1)     # 130 rows per A2A shard slot (2 heads x 65)

# Schraudolph exp: exp(0.125*s) ~= bitcast(int32(A*s + BEXP))
A_EXP = 0.125 * 1.4426950408889634 * (1 << 23)
B_EXP = (127.0 - 0.0430357) * (1 << 23)

_CACHE = {}
LAST_EXEC_NS = None
LAST_RESULTS = None


def _build():
    import concourse.tile as tile
    from concourse import bacc, mybir

    F32 = mybir.dt.float32
    BF16 = mybir.dt.bfloat16
    I32 = mybir.dt.int32
    AF = mybir.ActivationFunctionType
    ADD = mybir.AluOpType.add
    SUB = mybir.AluOpType.subtract
    MUL = mybir.AluOpType.mult

    nc = bacc.Bacc("TRN2", target_bir_lowering=False, debug=False,
                   num_devices=N_CORES)

    def inp(name, shape, dt):
        return nc.dram_tensor(name, shape, dt, kind="ExternalInput").ap()

    x_t = inp("x_t", [C, T], BF16)
    x_res = inp("x_res", [C, TCH], F32)
    w_qk = inp("w_qk", [128, CO, 512], BF16)
    b_qk = inp("b_qk", [128, 4], F32)
    w_v = inp("w_v", [128, CO, VD], BF16)
    b_v = inp("b_v", [1, VD], BF16)
    masks = inp("masks", [4, 128, 512], BF16)
    w_fc = inp("w_fc", [128, CO, FC], BF16)
    b_fc = inp("b_fc", [128, FO], F32)
    w_pj = inp("w_pj", [C, FC], BF16)      # row o*128+p, col kt*128+m
    b_pj = inp("b_pj", [128, CO], F32)
    s_fc = inp("s_fc", [128, FO], F32)    # column sums of w_fc (LN2 defer)
    sel0 = inp("sel0", [8, C], BF16)
    sel1 = inp("sel1", [8, C], BF16)
    ones_col = inp("ones_col", [128, 1], BF16)
    ones_row = inp("ones_row", [1, 128], BF16)
    v_ones = inp("v_ones", [128, TT, HPC, 1], BF16)

    out_t = nc.dram_tensor("out_t", [C, TCH], F32, kind="ExternalOutput").ap()

    with tile.TileContext(nc) as tc:
        # ---- persistent pools (enter order = reverse close order) ----
        const_cm = tc.tile_pool(name="const", bufs=1)
        dram_cm = tc.tile_pool(name="dram", bufs=1, space="DRAM")
        qkv_cm = tc.tile_pool(name="qkv", bufs=1)
        const = const_cm.__enter__()
        dram = dram_cm.__enter__()
        qkv_pool = qkv_cm.__enter__()

        ones_cb = const.tile([128, 1], BF16)      # stats reduce stationary
        ones_rb = const.tile([1, 128], BF16)      # broadcast stationary
        nc.sync.dma_start(ones_cb[:], ones_col)
        nc.sync.dma_start(ones_rb[:], ones_row)
        mask_sb = const.tile([128, 4, 512], BF16)
        sel_sb = [const.tile([8, C], BF16, name=f"sel{g}") for g in range(2)]
        bqk_sb = const.tile([128, 4], F32)
        bfc_sb = const.tile([128, FO], F32)
        bpj_sb = const.tile([128, CO], F32)
        sfc_sb = const.tile([128, FO], F32)
        nc.sync.dma_start(mask_sb[:], masks.rearrange("m p f -> p m f"))
        nc.sync.dma_start(sel_sb[0][:], sel0)
        nc.sync.dma_start(sel_sb[1][:], sel1)
        nc.sync.dma_start(bqk_sb[:], b_qk)
        nc.sync.dma_start(bfc_sb[:], b_fc)
        nc.sync.dma_start(bpj_sb[:], b_pj)
        nc.sync.dma_start(sfc_sb[:], s_fc)

        q_sb = qkv_pool.tile([128, 2, T], BF16)
        k_sb = qkv_pool.tile([128, 2, T], BF16)
        # per-head 128-wide slots: col 0:64 v, col 64 ones, 65:128 zero
        # (full 128-col stationary enables fast weight load for the AV mm)
        v_sb = qkv_pool.tile([128, TT, HPC, VS], BF16)
        nc.vector.memset(v_sb[:], 0.0)
        nc.sync.dma_start(v_sb[:, :, :, D:D + 1], v_ones)

        cc_in = [dram.tile([N_CORES * SLOT, TB], BF16, name=f"ccin{g}")
                 for g in range(2)]
        cc_out = [dram.tile([N_CORES * SLOT, TB], BF16, name=f"ccout{g}")
                  for g in range(2)]

        # ---- phase 1: load x, LN1, QKV -------------------------------
        with tc.tile_pool(name="xh", bufs=1) as xh_pool, \
             tc.tile_pool(name="ln_ps", bufs=1, space="PSUM") as ln_ps, \
             tc.tile_pool(name="bc_ps", bufs=2, space="PSUM") as bc_ps, \
             tc.tile_pool(name="rows", bufs=2) as rows, \
             tc.tile_pool(name="sq", bufs=3) as sq_pool, \
             tc.tile_pool(name="mm_ps", bufs=3, space="PSUM") as mm_ps:

            xh = xh_pool.tile([128, CO, T], BF16)
            wqk_sb = xh_pool.tile([128, CO, 512], BF16)
            wv_sb = xh_pool.tile([128, CO, VD], BF16)
            bv_row = xh_pool.tile([1, VD], BF16)
            xr = x_t.rearrange("(o p) t -> p o t", p=128)
            # chunked x load: chunk 0 + weights first so compute starts
            # as soon as ~2 MB (not 14 MB) has landed
            nc.sync.dma_start(xh[:, :, 0:512], xr[:, :, 0:512])
            nc.sync.dma_start(wqk_sb[:], w_qk)
            nc.sync.dma_start(wv_sb[:], w_v)
            nc.sync.dma_start(bv_row[:], b_v)
            for cn in range(1, NT):
                sl = slice(cn * 512, cn * 512 + 512)
                nc.sync.dma_start(xh[:, :, sl], xr[:, :, sl])

            # b_v broadcast to [128, VD]
            pbv = bc_ps.tile([128, 512], F32, tag="bc")
            nc.tensor.matmul(pbv[:, 0:VD], ones_rb[:],
                             bv_row[:], start=True, stop=True)
            bvbc_sb = xh_pool.tile([128, VD], BF16)
            nc.vector.tensor_copy(bvbc_sb[:], pbv[:, 0:VD])

            inv_c = 1.0 / C
            for cn in range(NT):
                sl = slice(cn * 512, cn * 512 + 512)
                ps_s = ln_ps.tile([1, 512], F32, tag="ps_s")
                ps_q = ln_ps.tile([1, 512], F32, tag="ps_q")
                for o in range(CO):
                    sq = sq_pool.tile([128, 512], BF16)
                    nc.vector.tensor_mul(sq[:], xh[:, o, sl], xh[:, o, sl])
                    nc.tensor.matmul(ps_s[:], ones_cb[:], xh[:, o, sl],
                                     start=(o == 0), stop=(o == CO - 1))
                    nc.tensor.matmul(ps_q[:], ones_cb[:], sq[:],
                                     start=(o == 0), stop=(o == CO - 1))
                mu = rows.tile([1, 512], F32, tag="mu")
                msq = rows.tile([1, 512], F32, tag="msq")
                var = rows.tile([1, 512], F32, tag="var")
                std = rows.tile([1, 512], F32, tag="std")
                rstd = rows.tile([1, 512], F32, tag="rstd")
                nc.vector.tensor_scalar_mul(mu[:], ps_s[:], inv_c)
                nc.vector.tensor_scalar_mul(msq[:], ps_q[:], inv_c)
                nc.vector.tensor_mul(var[:], mu[:], mu[:])
                nc.vector.tensor_tensor(var[:], msq[:], var[:], SUB)
                nc.vector.tensor_scalar_add(var[:], var[:], EPS)
                nc.scalar.activation(std[:], var[:], AF.Sqrt)
                nc.vector.reciprocal_approx_fast(rstd[:], std[:])
                rstd_bf = rows.tile([1, 512], BF16, tag="rstd_bf")
                nm_bf = rows.tile([1, 512], BF16, tag="nm_bf")
                nc.vector.tensor_copy(rstd_bf[:], rstd[:])
                nc.vector.scalar_tensor_tensor(nm_bf[:], mu[:], -1.0, rstd[:],
                                               MUL, MUL)

                pb = bc_ps.tile([128, 512], F32, tag="bc")
                nc.tensor.matmul(pb[:], ones_rb[:], rstd_bf[:],
                                 start=True, stop=True)
                rstd_bc = rows.tile([128, 512], BF16, tag="rstd_bc")
                nc.vector.tensor_copy(rstd_bc[:], pb[:])
                pb2 = bc_ps.tile([128, 512], F32, tag="bc")
                nc.tensor.matmul(pb2[:], ones_rb[:], nm_bf[:],
                                 start=True, stop=True)
                nm_bc = rows.tile([128, 512], BF16, tag="nm_bc")
                nc.vector.tensor_copy(nm_bc[:], pb2[:])

                for o in range(CO):
                    nc.vector.tensor_mul(xh[:, o, sl], xh[:, o, sl],
                                         rstd_bc[:])
                    nc.vector.tensor_add(xh[:, o, sl], xh[:, o, sl], nm_bc[:])

                # q, k for this chunk
                for m in range(4):
                    pq = mm_ps.tile([128, 512], F32, tag="mm")
                    for o in range(CO):
                        nc.tensor.matmul(pq[:],
                                         wqk_sb[:, o, m * 128:(m + 1) * 128],
                                         xh[:, o, sl],
                                         start=(o == 0), stop=(o == CO - 1))
                    dest = q_sb[:, m, sl] if m < 2 else k_sb[:, m - 2, sl]
                    nc.scalar.activation(dest, pq[:], AF.Identity,
                                         bias=bqk_sb[:, m:m + 1])

                # v (token-major) for the 4 token tiles of this chunk
                for tt in range(4 * cn, 4 * cn + 4):
                    tsl = slice(tt * 128, tt * 128 + 128)
                    pv_full = mm_ps.tile([128, 512], F32, tag="mm", name="pv")
                    pv = pv_full[:, 0:VD]
                    for o in range(CO):
                        nc.tensor.matmul(pv[:], xh[:, o, tsl], wv_sb[:, o, :],
                                         start=(o == 0), stop=(o == CO - 1))
                    vview = v_sb[:, tt, :, :]
                    nc.vector.tensor_tensor(
                        vview[:, :, 0:D],
                        pv[:].rearrange("p (h e) -> p h e", e=D),
                        bvbc_sb[:].rearrange("p (h e) -> p h e", e=D), ADD)

        # ---- phase 2+3: attention, one head-pair group per A2A -------
        # Heads are processed in pairs (2*pt, 2*pt+1): their 64-partition
        # score matmuls target distinct PE row groups and run concurrently,
        # and each pair shares a [128, 2, 512] PSUM tile so exp() batches
        # 1024 columns per ACT call.
        with tc.tile_pool(name="a", bufs=2) as a_pool, \
             tc.tile_pool(name="iexp", bufs=2) as i_pool, \
             tc.tile_pool(name="s_ps", bufs=3, space="PSUM") as s_ps, \
             tc.tile_pool(name="y_ps", bufs=2, space="PSUM") as y_ps, \
             tc.tile_pool(name="stage", bufs=18) as stage_pool:
            # 18 stage bufs: all 16 DMA queues freeze while a collective's
            # mesh data-plane is active, so the scatter DMAs back up; deep
            # staging lets attention compute run through the freeze

            for pt in range(2):
                for qb in range(NT):
                    qsl = slice(qb * 512, qb * 512 + 512)
                    nkv = 4 * qb + 4
                    a2 = a_pool.tile([128, TT, 2, 512], BF16, tag="a")
                    for t in range(nkv):
                        sp = s_ps.tile([128, 2, 512], F32)
                        nc.tensor.matmul(
                            sp[:, 0, :],
                            k_sb[0:64, pt, t * 128:(t + 1) * 128],
                            q_sb[0:64, pt, qsl],
                            start=True, stop=True)
                        nc.tensor.matmul(
                            sp[:, 1, :],
                            k_sb[64:128, pt, t * 128:(t + 1) * 128],
                            q_sb[64:128, pt, qsl],
                            start=True, stop=True)
                        diag = t >= 4 * qb
                        if not diag and (t % 4 == 1 or t % 8 == 6):
                            # Schraudolph exp on DVE (both heads at once)
                            it = i_pool.tile([128, 2, 512], I32)
                            nc.vector.tensor_scalar(
                                it[:], sp[:], A_EXP, B_EXP, MUL, ADD)
                            nc.vector.tensor_copy(a2[:, t, :, :],
                                                  it[:].bitcast(F32))
                        else:
                            nc.scalar.activation(a2[:, t, :, :], sp[:],
                                                 AF.Exp, scale=0.125)
                            if diag:
                                for s in range(2):
                                    nc.vector.tensor_mul(
                                        a2[:, t, s, :], a2[:, t, s, :],
                                        mask_sb[:, t - 4 * qb, :])
                    for s in range(2):
                        h = 2 * pt + s
                        py = y_ps.tile([D + 1, 512], F32)
                        for t in range(nkv):
                            nc.tensor.matmul(
                                py[:],
                                v_sb[:, t, h, 0:D + 1],
                                a2[:, t, s, :],
                                start=(t == 0), stop=(t == nkv - 1))
                        stg = stage_pool.tile([D + 1, 512], BF16, tag="stg")
                        nc.vector.tensor_copy(stg[:], py[:])
                        stg_last = stg
                        # scatter [65, 512] -> slots (2qb, 2qb+1), rows 65s
                        dst = cc_in[pt][:].rearrange(
                            "(j r) t -> r j t", j=N_CORES)[
                            65 * s:65 * s + 65, 2 * qb:2 * qb + 2, :]
                        nc.sync.dma_start(
                            dst, stg[:].rearrange("r (s2 t) -> r s2 t", s2=2))

        qkv_cm.__exit__(None, None, None)

        # ---- phase 4: y assemble, x2, LN2, MLP -----------------------
        # Group-0 assembly overlaps A2A#1 (it only needs cc_out[0]).
        with tc.tile_pool(name="mlp", bufs=1) as mlp_pool, \
             tc.tile_pool(name="ln2_ps", bufs=1, space="PSUM") as ln2_ps, \
             tc.tile_pool(name="bc2_ps", bufs=2, space="PSUM") as bc2_ps, \
             tc.tile_pool(name="rows2", bufs=1) as rows2, \
             tc.tile_pool(name="sq2", bufs=3) as sq2_pool, \
             tc.tile_pool(name="wp", bufs=2) as wp_pool, \
             tc.tile_pool(name="m_ps", bufs=3, space="PSUM") as m_ps, \
             tc.tile_pool(name="o_sb", bufs=2) as o_sb:

            wfc_sb = mlp_pool.tile([128, CO, FC], BF16)     # 8 MB
            x2 = mlp_pool.tile([128, CO, TCH], F32)
            nc.sync.dma_start(x2[:], x_res.rearrange("(o p) t -> p o t",
                                                     p=128))
            nc.sync.dma_start(wfc_sb[:], w_fc)
            y_sb = mlp_pool.tile([128, CO, TCH], BF16)
            x2bf = mlp_pool.tile([128, CO, TCH], BF16)
            stat_sb = [rows2.tile([1, TCH], F32, name=f"st{i}")
                       for i in range(4)]    # s0, q0, s1, q1

            def do_collective(pt):
                nc.gpsimd.collective_compute(
                    "AllToAll",
                    mybir.AluOpType.bypass,
                    replica_groups=[list(range(N_CORES))],
                    ins=[cc_in[pt].opt()],
                    outs=[cc_out[pt].opt()],
                )

            def assembly_dmas(pt, den_bf):
                # issued from the gpsimd queue: it already serializes on the
                # collective completion waits, so these sit exactly where
                # they become valid -- putting them on the sync queue would
                # block its FIFO (DMA issue + sem relays) on the collective
                src_all = cc_out[pt][:].rearrange(
                    "(bb g s r) t -> g s r bb t", bb=2, g=4, s=2, r=D + 1)
                for g4 in range(4):
                    o = 2 * g4 + pt
                    for s in range(2):
                        src = src_all[g4, s, 0:D, :, :]       # [64,2,256]
                        dst = y_sb[64 * s:64 * s + 64, o, :].rearrange(
                            "d (bb t) -> d bb t", bb=2)
                        nc.gpsimd.dma_start(dst, src)
                den_src = cc_out[pt][:].rearrange(
                    "(bb hh r) t -> hh r bb t", bb=2, hh=8, r=D + 1)[
                    :, D:D + 1, :, :]
                nc.gpsimd.dma_start(
                    den_bf[:].rearrange("hh (u bb t) -> hh u bb t",
                                        u=1, bb=2),
                    den_src)
            den_bfs = [rows2.tile([8, TCH], BF16, name=f"den{g}")
                       for g in range(2)]
            do_collective(0)
            assembly_dmas(0, den_bfs[0])
            do_collective(1)
            assembly_dmas(1, den_bfs[1])

            inv_c = 1.0 / C
            BYP = mybir.AluOpType.bypass
            for pt in range(2):
                den_bf = den_bfs[pt]
                den_f = rows2.tile([8, TCH], F32, tag="den_f")
                rr_f = rows2.tile([8, TCH], F32, tag="rr_f")
                rr_bf = rows2.tile([8, TCH], BF16, tag="rr_bf")
                # fence: fake-read pt1's last attention output (pt=0) /
                # group-0's last x2bf tile (pt=1) so the scheduler cannot
                # hoist this collective-gated chain ahead of live compute
                # on the DVE queue
                marker = (stg_last[0:8, :] if pt == 0
                          else x2bf[0:8, 6, :])
                nc.vector.scalar_tensor_tensor(den_f[:], den_bf[:], 1.0,
                                               marker, MUL, BYP)
                nc.vector.reciprocal_approx_fast(rr_f[:], den_f[:])
                nc.vector.tensor_copy(rr_bf[:], rr_f[:])

                ps2_s = ln2_ps.tile([1, TCH], F32, tag="s")
                ps2_q = ln2_ps.tile([1, TCH], F32, tag="q")
                for i, g4 in enumerate(range(4)):
                    o = 2 * g4 + pt
                    prr = bc2_ps.tile([128, TCH], F32, tag="bc2")
                    nc.tensor.matmul(prr[:],
                                     sel_sb[pt][:, o * 128:(o + 1) * 128],
                                     rr_bf[:], start=True, stop=True)
                    rrbc = o_sb.tile([128, TCH], BF16, tag="rrbc")
                    nc.vector.tensor_copy(rrbc[:], prr[:])
                    yn = o_sb.tile([128, TCH], F32, tag="yn")
                    nc.vector.tensor_mul(yn[:], y_sb[:, o, :], rrbc[:])
                    nc.vector.tensor_add(x2[:, o, :], x2[:, o, :], yn[:])
                    nc.vector.tensor_copy(x2bf[:, o, :], x2[:, o, :])
                    sq = sq2_pool.tile([128, TCH], BF16)
                    nc.vector.tensor_mul(sq[:], x2bf[:, o, :], x2bf[:, o, :])
                    nc.tensor.matmul(ps2_s[:], ones_cb[:], x2bf[:, o, :],
                                     start=(i == 0), stop=(i == 3))
                    nc.tensor.matmul(ps2_q[:], ones_cb[:], sq[:],
                                     start=(i == 0), stop=(i == 3))
                # move group stats to SBUF so the PSUM bank can be reused
                nc.vector.tensor_copy(stat_sb[2 * pt][:], ps2_s[:])
                nc.vector.tensor_copy(stat_sb[2 * pt + 1][:], ps2_q[:])

            # LN2 over the 512 on-core tokens
            mu2 = rows2.tile([1, TCH], F32, tag="r_mu2")
            msq2 = rows2.tile([1, TCH], F32, tag="r_msq2")
            var2 = rows2.tile([1, TCH], F32, tag="r_var2")
            nc.vector.tensor_tensor(mu2[:], stat_sb[0][:], stat_sb[2][:], ADD)
            nc.vector.tensor_tensor(msq2[:], stat_sb[1][:], stat_sb[3][:],
                                    ADD)
            nc.vector.tensor_scalar_mul(mu2[:], mu2[:], inv_c)
            nc.vector.tensor_scalar_mul(msq2[:], msq2[:], inv_c)
            nc.vector.tensor_mul(var2[:], mu2[:], mu2[:])
            nc.vector.tensor_tensor(var2[:], msq2[:], var2[:], SUB)
            nc.vector.tensor_scalar_add(var2[:], var2[:], EPS)
            std2 = rows2.tile([1, TCH], F32, tag="r_msq2")   # reuse dead buf
            nc.scalar.activation(std2[:], var2[:], AF.Sqrt)
            rstd2 = rows2.tile([1, TCH], F32, tag="r_var2")  # var2 dead too
            nc.vector.reciprocal_approx_fast(rstd2[:], std2[:])
            rstd2_bf = rows2.tile([1, TCH], BF16)
            nm2_bf = rows2.tile([1, TCH], BF16)
            nc.vector.tensor_copy(rstd2_bf[:], rstd2[:])
            nc.vector.scalar_tensor_tensor(nm2_bf[:], mu2[:], -1.0, rstd2[:],
                                           MUL, MUL)

            pb = bc2_ps.tile([128, TCH], F32, tag="bc2")
            nc.tensor.matmul(pb[:], ones_rb[:], rstd2_bf[:],
                             start=True, stop=True)
            rstd2_bc = rows2.tile([128, TCH], BF16)
            nc.vector.tensor_copy(rstd2_bc[:], pb[:])
            pb2 = bc2_ps.tile([128, TCH], F32, tag="bc2")
            nc.tensor.matmul(pb2[:], ones_rb[:], nm2_bf[:],
                             start=True, stop=True)
            nm2_bc = rows2.tile([128, TCH], BF16)
            nc.vector.tensor_copy(nm2_bc[:], pb2[:])

            # fc + gelu on RAW x2bf; the LN2 normalization is deferred into
            # a per-token scale/shift on the (otherwise idle) vector engine:
            #   fc(LN(x)) = rstd_t * fc_raw(x)_mt + (-mu*rstd)_t * S_m + b_m
            m_sb = mlp_pool.tile([128, FO, TCH], BF16)
            for mt in range(FO):
                pm = m_ps.tile([128, TCH], F32, tag="mm2")
                for o in range(CO):
                    nc.tensor.matmul(pm[:],
                                     wfc_sb[:, o, mt * 128:(mt + 1) * 128],
                                     x2bf[:, o, :],
                                     start=(o == 0), stop=(o == CO - 1))
                shift = o_sb.tile([128, TCH], BF16, tag="shift")
                nc.vector.tensor_scalar(shift[:], nm2_bc[:],
                                        sfc_sb[:, mt:mt + 1],
                                        bfc_sb[:, mt:mt + 1], MUL, ADD)
                tmp = o_sb.tile([128, TCH], BF16, tag="fcraw")
                nc.vector.tensor_mul(tmp[:], pm[:], rstd2_bc[:])
                nc.vector.tensor_add(tmp[:], tmp[:], shift[:])
                nc.scalar.activation(m_sb[:, mt, :], tmp[:], AF.Gelu)

            # proj + bias + residual (weights streamed per o-tile)
            out_r = out_t.rearrange("(o p) t -> p o t", p=128)
            wpj_r = w_pj.rearrange("(o p) (k m) -> o p k m", p=128, m=128)
            for o in range(CO):
                wt = wp_pool.tile([128, FO, 128], BF16, tag="wpj")
                nc.sync.dma_start(wt[:], wpj_r[o])
                pp = m_ps.tile([128, TCH], F32, tag="mm2")
                for kt in range(FO):
                    nc.tensor.matmul(pp[:], wt[:, kt, :], m_sb[:, kt, :],
                                     start=(kt == 0), stop=(kt == FO - 1))
                po_sb = o_sb.tile([128, TCH], F32, tag="po")
                nc.scalar.activation(po_sb[:], pp[:], AF.Identity,
                                     bias=bpj_sb[:, o:o + 1])
                fin = o_sb.tile([128, TCH], F32, tag="fin")
                nc.vector.tensor_add(fin[:], po_sb[:], x2[:, o, :])
                nc.sync.dma_start(out_r[:, o, :], fin[:])

        for cm in (dram_cm, const_cm):
            cm.__exit__(None, None, None)

    nc.compile()
    return nc


def _get_nc():
    if "nc" not in _CACHE:
        _CACHE["nc"] = _build()
    return _CACHE["nc"]


def _make_masks():
    m = np.zeros((4, 128, 512), np.float32)
    i = np.arange(128)[:, None]
    j = np.arange(512)[None, :]
    for t in range(4):
        m[t] = (128 * t + i <= j).astype(np.float32)
    return m


def kernel(x, ln1_g, ln1_b, W_attn, b_attn, ln2_g, ln2_b, W_fc, b_fc,
           W_proj, b_proj):
    global LAST_EXEC_NS, LAST_RESULTS
    import os
    import ml_dtypes

    from concourse.bass_utils import run_bass_kernel_spmd

    BF = ml_dtypes.bfloat16

    x = np.asarray(x, np.float32)
    W1 = np.asarray(ln1_g, np.float32)[:, None] * np.asarray(W_attn, np.float32)
    b1 = np.asarray(b_attn, np.float32) + \
        np.asarray(ln1_b, np.float32) @ np.asarray(W_attn, np.float32)
    Wf = np.asarray(ln2_g, np.float32)[:, None] * np.asarray(W_fc, np.float32)
    bf = np.asarray(b_fc, np.float32) + \
        np.asarray(ln2_b, np.float32) @ np.asarray(W_fc, np.float32)
    Wp = np.asarray(W_proj, np.float32)
    bp = np.asarray(b_proj, np.float32)

    masks = _make_masks().astype(BF)

    wfc_l = np.ascontiguousarray(
        Wf.reshape(CO, 128, FC).transpose(1, 0, 2)).astype(BF)
    # wt[p, kt, m] must equal Wp[kt*128+p, o*128+m]
    wpj_l = np.ascontiguousarray(
        Wp.reshape(FO, 128, CO, 128).transpose(2, 1, 0, 3).reshape(C, FC)
    ).astype(BF)
    bfc_l = np.ascontiguousarray(bf.reshape(FO, 128).T)
    bpj_l = np.ascontiguousarray(bp.reshape(CO, 128).T)
    sfc_l = np.ascontiguousarray(
        Wf.sum(axis=0).astype(np.float32).reshape(FO, 128).T)

    # per-group one-hot: rrbc[p, t] = rr[2*g4 + p//64, t] for o = 2*g4+pt
    sels = []
    for pt in range(2):
        sg = np.zeros((8, C), np.float32)
        for g4 in range(4):
            o = 2 * g4 + pt
            for s in range(2):
                sg[2 * g4 + s, o * 128 + 64 * s:o * 128 + 64 * s + 64] = 1.0
        sels.append(sg.astype(BF))

    xT = [np.ascontiguousarray(x[b].T) for b in range(B)]

    in_maps = []
    for c in range(N_CORES):
        b = c // 4
        g = c % 4
        qc = slice(g * HPC * D, (g + 1) * HPC * D)
        kc = slice(C + g * HPC * D, C + (g + 1) * HPC * D)
        vc = slice(2 * C + g * HPC * D, 2 * C + (g + 1) * HPC * D)
        wqk = np.concatenate([W1[:, qc], W1[:, kc]], axis=1)      # [1024,512]
        wv = W1[:, vc]                                            # [1024,256]
        tok0 = TB * c
        xres = np.ascontiguousarray(np.concatenate(
            [xT[0][:, tok0:tok0 + TB], xT[1][:, tok0:tok0 + TB]], axis=1))
        in_maps.append({
            "x_t": xT[b].astype(BF),
            "x_res": xres,
            "w_qk": np.ascontiguousarray(
                wqk.reshape(CO, 128, 512).transpose(1, 0, 2)).astype(BF),
            "b_qk": np.ascontiguousarray(
                np.concatenate([b1[qc], b1[kc]]).reshape(4, 128).T),
            "w_v": np.ascontiguousarray(
                wv.reshape(CO, 128, VD).transpose(1, 0, 2)).astype(BF),
            "b_v": np.ascontiguousarray(b1[vc][None, :]).astype(BF),
            "masks": masks,
            "w_fc": wfc_l,
            "b_fc": bfc_l,
            "w_pj": wpj_l,
            "b_pj": bpj_l,
            "s_fc": sfc_l,
            "sel0": sels[0],
            "sel1": sels[1],
            "ones_col": np.ones((128, 1), np.float32).astype(BF),
            "ones_row": np.ones((1, 128), np.float32).astype(BF),
            "v_ones": np.ones((128, TT, HPC, 1), np.float32).astype(BF),
        })

    nc = _get_nc()
    trace = os.environ.get("KERNEL_TRACE") == "1"
    kw = {}
    if trace:
        kw = dict(trace=True, trace_cores=list(range(N_CORES)))
    res = run_bass_kernel_spmd(nc, in_maps, core_ids=list(range(N_CORES)), **kw)
    LAST_EXEC_NS = res.exec_time_ns
    LAST_RESULTS = res

    out = np.empty((B, T, C), np.float32)
    for c in range(N_CORES):
        tok0 = TB * c
        r = res.results[c]["out_t"]
        out[0, tok0:tok0 + TB, :] = r[:, 0:TB].T
        out[1, tok0:tok0 + TB, :] = r[:, TB:2 * TB].T
    return out


# revision 43
# speedup vs baseline: 1.0301x; 1.0301x over previous
"""Trainium2 Bass kernel for a GPT-style transformer block (no attn out-proj).

Sharding (8 cores): attention is tensor-parallel over heads -- core c handles
batch c//4 and heads [4*(c%4), 4*(c%4)+4) over the full 2048-token causal
sequence. The MLP is token-parallel with a batch-interleaved assignment: core
r handles tokens [256*r, 256*r+256) of BOTH batches.

The feature->token reshard runs as TWO pipelined 8-rank AllToAlls, one
per head-pair group: A2A#0 ships heads {0,1} of each core's group while
the core computes heads {2,3}; the receive-side assembly (y normalize,
residual add, LN2 partial stats) for group 0 overlaps A2A#1. Both
collective calls sit after ALL scatter DMAs in program order so their
bounce-copy descriptors cannot block later compute DMAs on the rings.

Attention softmax normalization is deferred past the collective: each core
ships unnormalized sums (plus the denominator row, via an appended ones
column in V) and the receiving core divides after the exchange.

Matmuls run in bf16 (fp32 PSUM accumulate). exp() is split between the
scalar engine (table exp) and the vector engine (Schraudolph bit-trick
exp, ~3% max rel err, which washes out in softmax normalization).
"""

import numpy as np

B, T, C = 2, 2048, 1024
H, D = 16, 64
HPC = 4            # heads per core
VD = HPC * D       # 256 v features per core
N_CORES = 8
TCH = 512          # tokens per core for the MLP (256 from each batch)
TB = 256           # per-batch token block
EPS = 1e-5
FC = 4 * C
CO = C // 128      # 8
FO = FC // 128     # 32
NT = T // 512      # 4 query chunks
TT = T // 128      # 16 token tiles
VS = 128           # padded per-head v slot (64 v + 1 ones + 63 zero)
SLOT = 2 * (D + 1)     # 130 rows per A2A shard slot (2 heads x 65)

# Schraudolph exp: exp(0.125*s) ~= bitcast(int32(A*s + BEXP))
A_EXP = 0.125 * 1.4426950408889634 * (1 << 23)
B_EXP = (127.0 - 0.0430357) * (1 << 23)

_CACHE = {}
LAST_EXEC_NS = None
LAST_RESULTS = None


def _build():
    import concourse.tile as tile
    from concourse import bacc, mybir

    F32 = mybir.dt.float32
    BF16 = mybir.dt.bfloat16
    I32 = mybir.dt.int32
    AF = mybir.ActivationFunctionType
    ADD = mybir.AluOpType.add
    SUB = mybir.AluOpType.subtract
    MUL = mybir.AluOpType.mult

    nc = bacc.Bacc("TRN2", target_bir_lowering=False, debug=False,
                   num_devices=N_CORES)

    def inp(name, shape, dt):
        return nc.dram_tensor(name, shape, dt, kind="ExternalInput").ap()

    x_t = inp("x_t", [C, T], BF16)
    x_res = inp("x_res", [C, TCH], F32)
    w_qk = inp("w_qk", [128, CO, 512], BF16)
    b_qk = inp("b_qk", [128, 4], F32)
    w_v = inp("w_v", [128, CO, VD], BF16)
    b_v = inp("b_v", [1, VD], BF16)
    masks = inp("masks", [4, 128, 512], BF16)
    w_fc = inp("w_fc", [128, CO, FC], BF16)
    b_fc = inp("b_fc", [128, FO], F32)
    w_pj = inp("w_pj", [C, FC], BF16)      # row o*128+p, col kt*128+m
    b_pj = inp("b_pj", [128, CO], F32)
    s_fc = inp("s_fc", [128, FO], F32)    # column sums of w_fc (LN2 defer)
    sel0 = inp("sel0", [8, C], BF16)
    sel1 = inp("sel1", [8, C], BF16)
    ones_col = inp("ones_col", [128, 1], BF16)
    ones_row = inp("ones_row", [1, 128], BF16)
    v_ones = inp("v_ones", [128, TT, HPC, 1], BF16)

    out_t = nc.dram_tensor("out_t", [C, TCH], F32, kind="ExternalOutput").ap()

    with tile.TileContext(nc) as tc:
        # ---- persistent pools (enter order = reverse close order) ----
        const_cm = tc.tile_pool(name="const", bufs=1)
        dram_cm = tc.tile_pool(name="dram", bufs=1, space="DRAM")
        qkv_cm = tc.tile_pool(name="qkv", bufs=1)
        const = const_cm.__enter__()
        dram = dram_cm.__enter__()
        qkv_pool = qkv_cm.__enter__()

        ones_cb = const.tile([128, 1], BF16)      # stats reduce stationary
        ones_rb = const.tile([1, 128], BF16)      # broadcast stationary
        eps_sb = const.tile([1, 1], F32)          # LN epsilon (ACT bias)
        nc.vector.memset(eps_sb[:], EPS)
        nc.sync.dma_start(ones_cb[:], ones_col)
        nc.sync.dma_start(ones_rb[:], ones_row)
        mask_sb = const.tile([128, 4, 512], BF16)
        sel_sb = [const.tile([8, C], BF16, name=f"sel{g}") for g in range(2)]
        bqk_sb = const.tile([128, 4], F32)
        bfc_sb = const.tile([128, FO], F32)
        bpj_sb = const.tile([128, CO], F32)
        sfc_sb = const.tile([128, FO], F32)
        nc.sync.dma_start(mask_sb[:], masks.rearrange("m p f -> p m f"))
        nc.sync.dma_start(sel_sb[0][:], sel0)
        nc.sync.dma_start(sel_sb[1][:], sel1)
        nc.sync.dma_start(bqk_sb[:], b_qk)
        nc.sync.dma_start(bfc_sb[:], b_fc)
        nc.sync.dma_start(bpj_sb[:], b_pj)
        nc.sync.dma_start(sfc_sb[:], s_fc)

        q_sb = qkv_pool.tile([128, 2, T], BF16)
        k_sb = qkv_pool.tile([128, 2, T], BF16)
        # per-head 128-wide slots: col 0:64 v, col 64 ones, 65:128 zero
        # (full 128-col stationary enables fast weight load for the AV mm)
        v_sb = qkv_pool.tile([128, TT, HPC, VS], BF16)
        nc.vector.memset(v_sb[:], 0.0)
        nc.sync.dma_start(v_sb[:, :, :, D:D + 1], v_ones)

        cc_in = [dram.tile([N_CORES * SLOT, TB], BF16, name=f"ccin{g}")
                 for g in range(2)]
        cc_out = [dram.tile([N_CORES * SLOT, TB], BF16, name=f"ccout{g}")
                  for g in range(2)]

        # ---- phase 1+2: LN1/QKV interleaved with attention -----------
        # Two passes over the chunks, one per head pair. Pass P computes
        # q/k/v for head pair P of chunk cn, then immediately runs that
        # pair's attention for query block cn (its k/v prefix is complete).
        # Pass 0's last scatter lands ~20us earlier than a phase-major
        # order, so A2A#0's mesh finishes while pass 1 still computes.
        # Head-pair score matmuls target distinct PE row groups and run
        # concurrently; each pair shares a [128, 2, 512] PSUM tile so
        # exp() batches 1024 columns per ACT call.
        with tc.tile_pool(name="xh", bufs=1) as xh_pool, \
             tc.tile_pool(name="ln_ps", bufs=1, space="PSUM") as ln_ps, \
             tc.tile_pool(name="bc_ps", bufs=1, space="PSUM") as bc_ps, \
             tc.tile_pool(name="rows", bufs=2) as rows, \
             tc.tile_pool(name="sq", bufs=3) as sq_pool, \
             tc.tile_pool(name="mm_ps", bufs=2, space="PSUM") as mm_ps, \
             tc.tile_pool(name="a", bufs=2) as a_pool, \
             tc.tile_pool(name="iexp", bufs=2) as i_pool, \
             tc.tile_pool(name="y_ps", bufs=2, space="PSUM") as y_ps, \
             tc.tile_pool(name="stage", bufs=18) as stage_pool:
            # 18 stage bufs: all 16 DMA queues freeze while a collective's
            # mesh data-plane is active, so the scatter DMAs back up; deep
            # staging lets attention compute run through the freeze

            xh = xh_pool.tile([128, CO, T], BF16)
            wqk_sb = xh_pool.tile([128, CO, 512], BF16)
            wv_sb = xh_pool.tile([128, CO, VD], BF16)
            bv_row = xh_pool.tile([1, VD], BF16)
            xr = x_t.rearrange("(o p) t -> p o t", p=128)
            # chunked x load: chunk 0 + weights first so compute starts
            # as soon as ~2 MB (not 14 MB) has landed
            nc.sync.dma_start(xh[:, :, 0:512], xr[:, :, 0:512])
            nc.sync.dma_start(wqk_sb[:], w_qk)
            nc.sync.dma_start(wv_sb[:], w_v)
            nc.sync.dma_start(bv_row[:], b_v)
            for cn in range(1, NT):
                sl = slice(cn * 512, cn * 512 + 512)
                nc.sync.dma_start(xh[:, :, sl], xr[:, :, sl])

            # b_v broadcast to [128, VD]
            pbv = bc_ps.tile([128, 512], F32, tag="bc")
            nc.tensor.matmul(pbv[:, 0:VD], ones_rb[:],
                             bv_row[:], start=True, stop=True)
            bvbc_sb = xh_pool.tile([128, VD], BF16)
            nc.vector.tensor_copy(bvbc_sb[:], pbv[:, 0:VD])

            inv_c = 1.0 / C
            for P in range(2):
              for cn in range(NT):
                sl = slice(cn * 512, cn * 512 + 512)
                if P == 0:
                    # LN1 stats: sums and sumsqs share one PSUM bank at
                    # partitions 0 and 32 (distinct PE column groups)
                    ps_sq = ln_ps.tile([33, 512], F32, tag="ps")
                    ps_s = ps_sq[0:1, :]
                    ps_q = ps_sq[32:33, :]
                    for o in range(CO):
                        sq = sq_pool.tile([128, 512], BF16)
                        nc.vector.tensor_mul(sq[:], xh[:, o, sl],
                                             xh[:, o, sl])
                        nc.tensor.matmul(ps_s, ones_cb[:], xh[:, o, sl],
                                         start=(o == 0), stop=(o == CO - 1))
                        nc.tensor.matmul(ps_q, ones_cb[:], sq[:],
                                         start=(o == 0), stop=(o == CO - 1))
                    mu = rows.tile([1, 512], F32, tag="mu")
                    msq = rows.tile([1, 512], F32, tag="msq")
                    var = rows.tile([1, 512], F32, tag="var")
                    lnv = rows.tile([1, 512], F32, tag="lnv")
                    rstd = rows.tile([1, 512], F32, tag="rstd")
                    nc.vector.tensor_scalar_mul(mu[:], ps_s, inv_c)
                    nc.vector.tensor_scalar_mul(msq[:], ps_q, inv_c)
                    nc.vector.tensor_mul(var[:], mu[:], mu[:])
                    nc.vector.tensor_tensor(var[:], msq[:], var[:], SUB)
                    # rstd = exp(-0.5*ln(var+EPS)): Ln and Exp live in the
                    # same ACT table set as attention's exp -> no reloads
                    nc.scalar.activation(lnv[:], var[:], AF.Ln, bias=eps_sb[:])
                    nc.scalar.activation(rstd[:], lnv[:], AF.Exp, scale=-0.5)
                    rstd_bf = rows.tile([1, 512], BF16, tag="rstd_bf")
                    nm_bf = rows.tile([1, 512], BF16, tag="nm_bf")
                    nc.vector.tensor_copy(rstd_bf[:], rstd[:])
                    nc.vector.scalar_tensor_tensor(nm_bf[:], mu[:], -1.0,
                                                   rstd[:], MUL, MUL)

                    pb = bc_ps.tile([128, 512], F32, tag="bc")
                    nc.tensor.matmul(pb[:], ones_rb[:], rstd_bf[:],
                                     start=True, stop=True)
                    rstd_bc = rows.tile([128, 512], BF16, tag="rstd_bc")
                    nc.vector.tensor_copy(rstd_bc[:], pb[:])
                    pb2 = bc_ps.tile([128, 512], F32, tag="bc")
                    nc.tensor.matmul(pb2[:], ones_rb[:], nm_bf[:],
                                     start=True, stop=True)
                    nm_bc = rows.tile([128, 512], BF16, tag="nm_bc")
                    nc.vector.tensor_copy(nm_bc[:], pb2[:])

                    for o in range(CO):
                        nc.vector.tensor_mul(xh[:, o, sl], xh[:, o, sl],
                                             rstd_bc[:])
                        nc.vector.tensor_add(xh[:, o, sl], xh[:, o, sl],
                                             nm_bc[:])

                # q, k for head pair P of this chunk (m=P: q, m=P+2: k)
                for m in (P, P + 2):
                    pq_full = mm_ps.tile([128, 2, 512], F32, tag="mm")
                    pq = pq_full[:, 0, :]
                    for o in range(CO):
                        nc.tensor.matmul(pq,
                                         wqk_sb[:, o, m * 128:(m + 1) * 128],
                                         xh[:, o, sl],
                                         start=(o == 0), stop=(o == CO - 1))
                    dest = q_sb[:, P, sl] if m < 2 else k_sb[:, P, sl]
                    nc.scalar.activation(dest, pq, AF.Identity,
                                         bias=bqk_sb[:, m:m + 1])

                # v (token-major) for heads (2P, 2P+1) of this chunk
                vcl = slice(128 * P, 128 * P + 128)
                for tt in range(4 * cn, 4 * cn + 4):
                    tsl = slice(tt * 128, tt * 128 + 128)
                    pv_full = mm_ps.tile([128, 2, 512], F32, tag="mm",
                                         name="pv")
                    pv = pv_full[:, 0, 0:128]
                    for o in range(CO):
                        nc.tensor.matmul(pv, xh[:, o, tsl],
                                         wv_sb[:, o, vcl],
                                         start=(o == 0), stop=(o == CO - 1))
                    vview = v_sb[:, tt, 2 * P:2 * P + 2, :]
                    nc.vector.tensor_tensor(
                        vview[:, :, 0:D],
                        pv.rearrange("p (h e) -> p h e", e=D),
                        bvbc_sb[:, vcl].rearrange("p (h e) -> p h e", e=D),
                        ADD)

                # attention for pair P, query block cn
                pt, qb = P, cn
                qsl = sl
                nkv = 4 * qb + 4
                a2 = a_pool.tile([128, TT, 2, 512], BF16, tag="a")
                for t in range(nkv):
                    sp = mm_ps.tile([128, 2, 512], F32, tag="mm", name="sp")
                    nc.tensor.matmul(
                        sp[:, 0, :],
                        k_sb[0:64, pt, t * 128:(t + 1) * 128],
                        q_sb[0:64, pt, qsl],
                        start=True, stop=True)
                    nc.tensor.matmul(
                        sp[:, 1, :],
                        k_sb[64:128, pt, t * 128:(t + 1) * 128],
                        q_sb[64:128, pt, qsl],
                        start=True, stop=True)
                    diag = t >= 4 * qb
                    if not diag and (t % 4 == 1 or t % 8 == 6):
                        # Schraudolph exp on DVE (both heads at once)
                        it = i_pool.tile([128, 2, 512], I32)
                        nc.vector.tensor_scalar(
                            it[:], sp[:], A_EXP, B_EXP, MUL, ADD)
                        nc.vector.tensor_copy(a2[:, t, :, :],
                                              it[:].bitcast(F32))
                    else:
                        nc.scalar.activation(a2[:, t, :, :], sp[:],
                                             AF.Exp, scale=0.125)
                        if diag:
                            for s in range(2):
                                nc.vector.tensor_mul(
                                    a2[:, t, s, :], a2[:, t, s, :],
                                    mask_sb[:, t - 4 * qb, :])
                for s in range(2):
                    h = 2 * pt + s
                    py = y_ps.tile([D + 1, 512], F32)
                    for t in range(nkv):
                        nc.tensor.matmul(
                            py[:],
                            v_sb[:, t, h, 0:D + 1],
                            a2[:, t, s, :],
                            start=(t == 0), stop=(t == nkv - 1))
                    stg = stage_pool.tile([D + 1, 512], BF16, tag="stg")
                    nc.vector.tensor_copy(stg[:], py[:])
                    stg_last = stg
                    # scatter [65, 512] -> slots (2qb, 2qb+1), rows 65s
                    dst = cc_in[pt][:].rearrange(
                        "(j r) t -> r j t", j=N_CORES)[
                        65 * s:65 * s + 65, 2 * qb:2 * qb + 2, :]
                    nc.sync.dma_start(
                        dst, stg[:].rearrange("r (s2 t) -> r s2 t", s2=2))

        qkv_cm.__exit__(None, None, None)

        # ---- phase 4: y assemble, x2, LN2, MLP -----------------------
        # Group-0 assembly overlaps A2A#1 (it only needs cc_out[0]).
        with tc.tile_pool(name="mlp", bufs=1) as mlp_pool, \
             tc.tile_pool(name="ln2_ps", bufs=1, space="PSUM") as ln2_ps, \
             tc.tile_pool(name="bc2_ps", bufs=2, space="PSUM") as bc2_ps, \
             tc.tile_pool(name="rows2", bufs=1) as rows2, \
             tc.tile_pool(name="sq2", bufs=3) as sq2_pool, \
             tc.tile_pool(name="wp", bufs=2) as wp_pool, \
             tc.tile_pool(name="m_ps", bufs=3, space="PSUM") as m_ps, \
             tc.tile_pool(name="o_sb", bufs=2) as o_sb:

            wfc_sb = mlp_pool.tile([128, CO, FC], BF16)     # 8 MB
            x2 = mlp_pool.tile([128, CO, TCH], F32)
            nc.sync.dma_start(x2[:], x_res.rearrange("(o p) t -> p o t",
                                                     p=128))
            nc.sync.dma_start(wfc_sb[:], w_fc)
            y_sb = mlp_pool.tile([128, CO, TCH], BF16)
            x2bf = mlp_pool.tile([128, CO, TCH], BF16)
            stat_sb = [rows2.tile([1, TCH], F32, name=f"st{i}")
                       for i in range(4)]    # s0, q0, s1, q1

            def do_collective(pt):
                nc.gpsimd.collective_compute(
                    "AllToAll",
                    mybir.AluOpType.bypass,
                    replica_groups=[list(range(N_CORES))],
                    ins=[cc_in[pt].opt()],
                    outs=[cc_out[pt].opt()],
                )

            def assembly_dmas(pt, den_bf):
                # issued from the gpsimd queue: it already serializes on the
                # collective completion waits, so these sit exactly where
                # they become valid -- putting them on the sync queue would
                # block its FIFO (DMA issue + sem relays) on the collective
                src_all = cc_out[pt][:].rearrange(
                    "(bb g s r) t -> g s r bb t", bb=2, g=4, s=2, r=D + 1)
                for g4 in range(4):
                    o = 2 * g4 + pt
                    for s in range(2):
                        src = src_all[g4, s, 0:D, :, :]       # [64,2,256]
                        dst = y_sb[64 * s:64 * s + 64, o, :].rearrange(
                            "d (bb t) -> d bb t", bb=2)
                        nc.gpsimd.dma_start(dst, src)
                den_src = cc_out[pt][:].rearrange(
                    "(bb hh r) t -> hh r bb t", bb=2, hh=8, r=D + 1)[
                    :, D:D + 1, :, :]
                nc.gpsimd.dma_start(
                    den_bf[:].rearrange("hh (u bb t) -> hh u bb t",
                                        u=1, bb=2),
                    den_src)
            den_bfs = [rows2.tile([8, TCH], BF16, name=f"den{g}")
                       for g in range(2)]
            do_collective(0)
            assembly_dmas(0, den_bfs[0])
            do_collective(1)
            assembly_dmas(1, den_bfs[1])

            inv_c = 1.0 / C
            BYP = mybir.AluOpType.bypass
            for pt in range(2):
                den_bf = den_bfs[pt]
                den_f = rows2.tile([8, TCH], F32, tag="den_f")
                rr_f = rows2.tile([8, TCH], F32, tag="rr_f")
                rr_bf = rows2.tile([8, TCH], BF16, tag="rr_bf")
                # fence: fake-read pt1's last attention output (pt=0) /
                # group-0's last x2bf tile (pt=1) so the scheduler cannot
                # hoist this collective-gated chain ahead of live compute
                # on the DVE queue
                marker = (stg_last[0:8, :] if pt == 0
                          else x2bf[0:8, 6, :])
                nc.vector.scalar_tensor_tensor(den_f[:], den_bf[:], 1.0,
                                               marker, MUL, BYP)
                nc.vector.reciprocal_approx_fast(rr_f[:], den_f[:])
                nc.vector.tensor_copy(rr_bf[:], rr_f[:])

                ps2_sq = ln2_ps.tile([33, TCH], F32, tag="sq")
                ps2_s = ps2_sq[0:1, :]
                ps2_q = ps2_sq[32:33, :]
                for i, g4 in enumerate(range(4)):
                    o = 2 * g4 + pt
                    prr = bc2_ps.tile([128, TCH], F32, tag="bc2")
                    nc.tensor.matmul(prr[:],
                                     sel_sb[pt][:, o * 128:(o + 1) * 128],
                                     rr_bf[:], start=True, stop=True)
                    rrbc = o_sb.tile([128, TCH], BF16, tag="rrbc")
                    nc.vector.tensor_copy(rrbc[:], prr[:])
                    yn = o_sb.tile([128, TCH], F32, tag="yn")
                    nc.vector.tensor_mul(yn[:], y_sb[:, o, :], rrbc[:])
                    nc.vector.tensor_add(x2[:, o, :], x2[:, o, :], yn[:])
                    nc.vector.tensor_copy(x2bf[:, o, :], x2[:, o, :])
                    sq = sq2_pool.tile([128, TCH], BF16)
                    nc.vector.tensor_mul(sq[:], x2bf[:, o, :], x2bf[:, o, :])
                    nc.tensor.matmul(ps2_s, ones_cb[:], x2bf[:, o, :],
                                     start=(i == 0), stop=(i == 3))
                    nc.tensor.matmul(ps2_q, ones_cb[:], sq[:],
                                     start=(i == 0), stop=(i == 3))
                # move group stats to SBUF so the PSUM bank can be reused
                nc.vector.tensor_copy(stat_sb[2 * pt][:], ps2_s)
                nc.vector.tensor_copy(stat_sb[2 * pt + 1][:], ps2_q)

            # LN2 over the 512 on-core tokens
            mu2 = rows2.tile([1, TCH], F32, tag="r_mu2")
            msq2 = rows2.tile([1, TCH], F32, tag="r_msq2")
            var2 = rows2.tile([1, TCH], F32, tag="r_var2")
            nc.vector.tensor_tensor(mu2[:], stat_sb[0][:], stat_sb[2][:], ADD)
            nc.vector.tensor_tensor(msq2[:], stat_sb[1][:], stat_sb[3][:],
                                    ADD)
            nc.vector.tensor_scalar_mul(mu2[:], mu2[:], inv_c)
            nc.vector.tensor_scalar_mul(msq2[:], msq2[:], inv_c)
            nc.vector.tensor_mul(var2[:], mu2[:], mu2[:])
            nc.vector.tensor_tensor(var2[:], msq2[:], var2[:], SUB)
            lnv2 = rows2.tile([1, TCH], F32, tag="r_msq2")   # reuse dead buf
            nc.scalar.activation(lnv2[:], var2[:], AF.Ln, bias=eps_sb[:])
            rstd2 = rows2.tile([1, TCH], F32, tag="r_var2")  # var2 dead too
            nc.scalar.activation(rstd2[:], lnv2[:], AF.Exp, scale=-0.5)
            rstd2_bf = rows2.tile([1, TCH], BF16)
            nm2_bf = rows2.tile([1, TCH], BF16)
            nc.vector.tensor_copy(rstd2_bf[:], rstd2[:])
            nc.vector.scalar_tensor_tensor(nm2_bf[:], mu2[:], -1.0, rstd2[:],
                                           MUL, MUL)

            pb = bc2_ps.tile([128, TCH], F32, tag="bc2")
            nc.tensor.matmul(pb[:], ones_rb[:], rstd2_bf[:],
                             start=True, stop=True)
            rstd2_bc = rows2.tile([128, TCH], BF16)
            nc.vector.tensor_copy(rstd2_bc[:], pb[:])
            pb2 = bc2_ps.tile([128, TCH], F32, tag="bc2")
            nc.tensor.matmul(pb2[:], ones_rb[:], nm2_bf[:],
                             start=True, stop=True)
            nm2_bc = rows2.tile([128, TCH], BF16)
            nc.vector.tensor_copy(nm2_bc[:], pb2[:])

            # fc + gelu on RAW x2bf; the LN2 normalization is deferred into
            # a per-token scale/shift on the (otherwise idle) vector engine:
            #   fc(LN(x)) = rstd_t * fc_raw(x)_mt + (-mu*rstd)_t * S_m + b_m
            m_sb = mlp_pool.tile([128, FO, TCH], BF16)
            for mt in range(FO):
                pm = m_ps.tile([128, TCH], F32, tag="mm2")
                for o in range(CO):
                    nc.tensor.matmul(pm[:],
                                     wfc_sb[:, o, mt * 128:(mt + 1) * 128],
                                     x2bf[:, o, :],
                                     start=(o == 0), stop=(o == CO - 1))
                shift = o_sb.tile([128, TCH], BF16, tag="shift")
                nc.vector.tensor_scalar(shift[:], nm2_bc[:],
                                        sfc_sb[:, mt:mt + 1],
                                        bfc_sb[:, mt:mt + 1], MUL, ADD)
                tmp = o_sb.tile([128, TCH], BF16, tag="fcraw")
                nc.vector.tensor_mul(tmp[:], pm[:], rstd2_bc[:])
                nc.vector.tensor_add(tmp[:], tmp[:], shift[:])
                nc.scalar.activation(m_sb[:, mt, :], tmp[:], AF.Gelu)

            # proj + bias + residual (weights streamed per o-tile)
            out_r = out_t.rearrange("(o p) t -> p o t", p=128)
            wpj_r = w_pj.rearrange("(o p) (k m) -> o p k m", p=128, m=128)
            for o in range(CO):
                wt = wp_pool.tile([128, FO, 128], BF16, tag="wpj")
                nc.sync.dma_start(wt[:], wpj_r[o])
                pp = m_ps.tile([128, TCH], F32, tag="mm2")
                for kt in range(FO):
                    nc.tensor.matmul(pp[:], wt[:, kt, :], m_sb[:, kt, :],
                                     start=(kt == 0), stop=(kt == FO - 1))
                po_sb = o_sb.tile([128, TCH], F32, tag="po")
                nc.scalar.activation(po_sb[:], pp[:], AF.Identity,
                                     bias=bpj_sb[:, o:o + 1])
                fin = o_sb.tile([128, TCH], F32, tag="fin")
                nc.vector.tensor_add(fin[:], po_sb[:], x2[:, o, :])
                nc.sync.dma_start(out_r[:, o, :], fin[:])

        for cm in (dram_cm, const_cm):
            cm.__exit__(None, None, None)

    nc.compile()
    return nc


def _get_nc():
    if "nc" not in _CACHE:
        _CACHE["nc"] = _build()
    return _CACHE["nc"]


def _make_masks():
    m = np.zeros((4, 128, 512), np.float32)
    i = np.arange(128)[:, None]
    j = np.arange(512)[None, :]
    for t in range(4):
        m[t] = (128 * t + i <= j).astype(np.float32)
    return m


def kernel(x, ln1_g, ln1_b, W_attn, b_attn, ln2_g, ln2_b, W_fc, b_fc,
           W_proj, b_proj):
    global LAST_EXEC_NS, LAST_RESULTS
    import os
    import ml_dtypes

    from concourse.bass_utils import run_bass_kernel_spmd

    BF = ml_dtypes.bfloat16

    x = np.asarray(x, np.float32)
    W1 = np.asarray(ln1_g, np.float32)[:, None] * np.asarray(W_attn, np.float32)
    b1 = np.asarray(b_attn, np.float32) + \
        np.asarray(ln1_b, np.float32) @ np.asarray(W_attn, np.float32)
    Wf = np.asarray(ln2_g, np.float32)[:, None] * np.asarray(W_fc, np.float32)
    bf = np.asarray(b_fc, np.float32) + \
        np.asarray(ln2_b, np.float32) @ np.asarray(W_fc, np.float32)
    Wp = np.asarray(W_proj, np.float32)
    bp = np.asarray(b_proj, np.float32)

    masks = _make_masks().astype(BF)

    wfc_l = np.ascontiguousarray(
        Wf.reshape(CO, 128, FC).transpose(1, 0, 2)).astype(BF)
    # wt[p, kt, m] must equal Wp[kt*128+p, o*128+m]
    wpj_l = np.ascontiguousarray(
        Wp.reshape(FO, 128, CO, 128).transpose(2, 1, 0, 3).reshape(C, FC)
    ).astype(BF)
    bfc_l = np.ascontiguousarray(bf.reshape(FO, 128).T)
    bpj_l = np.ascontiguousarray(bp.reshape(CO, 128).T)
    sfc_l = np.ascontiguousarray(
        Wf.sum(axis=0).astype(np.float32).reshape(FO, 128).T)

    # per-group one-hot: rrbc[p, t] = rr[2*g4 + p//64, t] for o = 2*g4+pt
    sels = []
    for pt in range(2):
        sg = np.zeros((8, C), np.float32)
        for g4 in range(4):
            o = 2 * g4 + pt
            for s in range(2):
                sg[2 * g4 + s, o * 128 + 64 * s:o * 128 + 64 * s + 64] = 1.0
        sels.append(sg.astype(BF))

    xT = [np.ascontiguousarray(x[b].T) for b in range(B)]

    in_maps = []
    for c in range(N_CORES):
        b = c // 4
        g = c % 4
        qc = slice(g * HPC * D, (g + 1) * HPC * D)
        kc = slice(C + g * HPC * D, C + (g + 1) * HPC * D)
        vc = slice(2 * C + g * HPC * D, 2 * C + (g + 1) * HPC * D)
        wqk = np.concatenate([W1[:, qc], W1[:, kc]], axis=1)      # [1024,512]
        wv = W1[:, vc]                                            # [1024,256]
        tok0 = TB * c
        xres = np.ascontiguousarray(np.concatenate(
            [xT[0][:, tok0:tok0 + TB], xT[1][:, tok0:tok0 + TB]], axis=1))
        in_maps.append({
            "x_t": xT[b].astype(BF),
            "x_res": xres,
            "w_qk": np.ascontiguousarray(
                wqk.reshape(CO, 128, 512).transpose(1, 0, 2)).astype(BF),
            "b_qk": np.ascontiguousarray(
                np.concatenate([b1[qc], b1[kc]]).reshape(4, 128).T),
            "w_v": np.ascontiguousarray(
                wv.reshape(CO, 128, VD).transpose(1, 0, 2)).astype(BF),
            "b_v": np.ascontiguousarray(b1[vc][None, :]).astype(BF),
            "masks": masks,
            "w_fc": wfc_l,
            "b_fc": bfc_l,
            "w_pj": wpj_l,
            "b_pj": bpj_l,
            "s_fc": sfc_l,
            "sel0": sels[0],
            "sel1": sels[1],
            "ones_col": np.ones((128, 1), np.float32).astype(BF),
            "ones_row": np.ones((1, 128), np.float32).astype(BF),
            "v_ones": np.ones((128, TT, HPC, 1), np.float32).astype(BF),
        })

    nc = _get_nc()
    trace = os.environ.get("KERNEL_TRACE") == "1"
    kw = {}
    if trace:
        kw = dict(trace=True, trace_cores=list(range(N_CORES)))
    res = run_bass_kernel_spmd(nc, in_maps, core_ids=list(range(N_CORES)), **kw)
    LAST_EXEC_NS = res.exec_time_ns
    LAST_RESULTS = res

    out = np.empty((B, T, C), np.float32)
    for c in range(N_CORES):
        tok0 = TB * c
        r = res.results[c]["out_t"]
        out[0, tok0:tok0 + TB, :] = r[:, 0:TB].T
        out[1, tok0:tok0 + TB, :] = r[:, TB:2 * TB].T
    return out


# revision 44
# speedup vs baseline: 1.0313x; 1.0012x over previous
"""Trainium2 Bass kernel for a GPT-style transformer block (no attn out-proj).

Sharding (8 cores): attention is tensor-parallel over heads -- core c handles
batch c//4 and heads [4*(c%4), 4*(c%4)+4) over the full 2048-token causal
sequence. The MLP is token-parallel with a batch-interleaved assignment: core
r handles tokens [256*r, 256*r+256) of BOTH batches.

The feature->token reshard runs as TWO pipelined 8-rank AllToAlls, one
per head-pair group: A2A#0 ships heads {0,1} of each core's group while
the core computes heads {2,3}; the receive-side assembly (y normalize,
residual add, LN2 partial stats) for group 0 overlaps A2A#1. Both
collective calls sit after ALL scatter DMAs in program order so their
bounce-copy descriptors cannot block later compute DMAs on the rings.

Attention softmax normalization is deferred past the collective: each core
ships unnormalized sums (plus the denominator row, via an appended ones
column in V) and the receiving core divides after the exchange.

Matmuls run in bf16 (fp32 PSUM accumulate). exp() is split between the
scalar engine (table exp) and the vector engine (Schraudolph bit-trick
exp, ~3% max rel err, which washes out in softmax normalization).
"""

import numpy as np

B, T, C = 2, 2048, 1024
H, D = 16, 64
HPC = 4            # heads per core
VD = HPC * D       # 256 v features per core
N_CORES = 8
TCH = 512          # tokens per core for the MLP (256 from each batch)
TB = 256           # per-batch token block
EPS = 1e-5
FC = 4 * C
CO = C // 128      # 8
FO = FC // 128     # 32
NT = T // 512      # 4 query chunks
TT = T // 128      # 16 token tiles
VS = 128           # padded per-head v slot (64 v + 1 ones + 63 zero)
SLOT = 2 * (D + 1)     # 130 rows per A2A shard slot (2 heads x 65)

# Schraudolph exp: exp(0.125*s) ~= bitcast(int32(A*s + BEXP))
A_EXP = 0.125 * 1.4426950408889634 * (1 << 23)
B_EXP = (127.0 - 0.0430357) * (1 << 23)

_CACHE = {}
LAST_EXEC_NS = None
LAST_RESULTS = None


def _build():
    import concourse.tile as tile
    from concourse import bacc, mybir

    F32 = mybir.dt.float32
    BF16 = mybir.dt.bfloat16
    I32 = mybir.dt.int32
    AF = mybir.ActivationFunctionType
    ADD = mybir.AluOpType.add
    SUB = mybir.AluOpType.subtract
    MUL = mybir.AluOpType.mult

    nc = bacc.Bacc("TRN2", target_bir_lowering=False, debug=False,
                   num_devices=N_CORES)

    def inp(name, shape, dt):
        return nc.dram_tensor(name, shape, dt, kind="ExternalInput").ap()

    x_t = inp("x_t", [C, T], BF16)
    x_res = inp("x_res", [C, TCH], F32)
    w_qk = inp("w_qk", [128, CO, 512], BF16)
    b_qk = inp("b_qk", [128, 4], F32)
    w_v = inp("w_v", [128, CO, VD], BF16)
    b_v = inp("b_v", [1, VD], BF16)
    masks = inp("masks", [4, 128, 512], BF16)
    w_fc = inp("w_fc", [128, CO, FC], BF16)
    b_fc = inp("b_fc", [128, FO], F32)
    w_pj = inp("w_pj", [C, FC], BF16)      # row o*128+p, col kt*128+m
    b_pj = inp("b_pj", [128, CO], F32)
    s_fc = inp("s_fc", [128, FO], F32)    # column sums of w_fc (LN2 defer)
    sel0 = inp("sel0", [8, C], BF16)
    sel1 = inp("sel1", [8, C], BF16)
    ones_col = inp("ones_col", [128, 1], BF16)
    ones_row = inp("ones_row", [1, 128], BF16)
    v_ones = inp("v_ones", [128, TT, HPC, 1], BF16)

    out_t = nc.dram_tensor("out_t", [C, TCH], F32, kind="ExternalOutput").ap()

    with tile.TileContext(nc) as tc:
        # ---- persistent pools (enter order = reverse close order) ----
        const_cm = tc.tile_pool(name="const", bufs=1)
        dram_cm = tc.tile_pool(name="dram", bufs=1, space="DRAM")
        qkv_cm = tc.tile_pool(name="qkv", bufs=1)
        const = const_cm.__enter__()
        dram = dram_cm.__enter__()
        qkv_pool = qkv_cm.__enter__()

        ones_cb = const.tile([128, 1], BF16)      # stats reduce stationary
        ones_rb = const.tile([1, 128], BF16)      # broadcast stationary
        eps_sb = const.tile([1, 1], F32)          # LN epsilon (ACT bias)
        nc.vector.memset(eps_sb[:], EPS)
        nc.sync.dma_start(ones_cb[:], ones_col)
        nc.sync.dma_start(ones_rb[:], ones_row)
        mask_sb = const.tile([128, 4, 512], BF16)
        sel_sb = [const.tile([8, C], BF16, name=f"sel{g}") for g in range(2)]
        bqk_sb = const.tile([128, 4], F32)
        bfc_sb = const.tile([128, FO], F32)
        bpj_sb = const.tile([128, CO], F32)
        sfc_sb = const.tile([128, FO], F32)
        nc.sync.dma_start(mask_sb[:], masks.rearrange("m p f -> p m f"))
        nc.sync.dma_start(sel_sb[0][:], sel0)
        nc.sync.dma_start(sel_sb[1][:], sel1)
        nc.sync.dma_start(bqk_sb[:], b_qk)
        nc.sync.dma_start(bfc_sb[:], b_fc)
        nc.sync.dma_start(bpj_sb[:], b_pj)
        nc.sync.dma_start(sfc_sb[:], s_fc)

        q_sb = qkv_pool.tile([128, 2, T], BF16)
        k_sb = qkv_pool.tile([128, 2, T], BF16)
        # per-head 128-wide slots: col 0:64 v, col 64 ones, 65:128 zero
        # (full 128-col stationary enables fast weight load for the AV mm)
        v_sb = qkv_pool.tile([128, TT, HPC, VS], BF16)
        nc.vector.memset(v_sb[:], 0.0)
        nc.sync.dma_start(v_sb[:, :, :, D:D + 1], v_ones)

        cc_in = [dram.tile([N_CORES * SLOT, TB], BF16, name=f"ccin{g}")
                 for g in range(2)]
        cc_out = [dram.tile([N_CORES * SLOT, TB], BF16, name=f"ccout{g}")
                  for g in range(2)]

        # ---- phase 1+2: LN1/QKV interleaved with attention -----------
        # Two passes over the chunks, one per head pair. Pass P computes
        # q/k/v for head pair P of chunk cn, then immediately runs that
        # pair's attention for query block cn (its k/v prefix is complete).
        # Pass 0's last scatter lands ~20us earlier than a phase-major
        # order, so A2A#0's mesh finishes while pass 1 still computes.
        # Head-pair score matmuls target distinct PE row groups and run
        # concurrently; each pair shares a [128, 2, 512] PSUM tile so
        # exp() batches 1024 columns per ACT call.
        with tc.tile_pool(name="xh", bufs=1) as xh_pool, \
             tc.tile_pool(name="ln_ps", bufs=1, space="PSUM") as ln_ps, \
             tc.tile_pool(name="bc_ps", bufs=1, space="PSUM") as bc_ps, \
             tc.tile_pool(name="rows", bufs=2) as rows, \
             tc.tile_pool(name="sq", bufs=3) as sq_pool, \
             tc.tile_pool(name="mm_ps", bufs=2, space="PSUM") as mm_ps, \
             tc.tile_pool(name="a", bufs=2) as a_pool, \
             tc.tile_pool(name="iexp", bufs=2) as i_pool, \
             tc.tile_pool(name="y_ps", bufs=2, space="PSUM") as y_ps, \
             tc.tile_pool(name="stage", bufs=18) as stage_pool:
            # 18 stage bufs: all 16 DMA queues freeze while a collective's
            # mesh data-plane is active, so the scatter DMAs back up; deep
            # staging lets attention compute run through the freeze

            xh = xh_pool.tile([128, CO, T], BF16)
            wqk_sb = xh_pool.tile([128, CO, 512], BF16)
            wv_sb = xh_pool.tile([128, CO, VD], BF16)
            bv_row = xh_pool.tile([1, VD], BF16)
            xr = x_t.rearrange("(o p) t -> p o t", p=128)
            # chunked x load: chunk 0 + weights first so compute starts
            # as soon as ~2 MB (not 14 MB) has landed
            nc.sync.dma_start(xh[:, :, 0:512], xr[:, :, 0:512])
            nc.sync.dma_start(wqk_sb[:], w_qk)
            nc.sync.dma_start(wv_sb[:], w_v)
            nc.sync.dma_start(bv_row[:], b_v)
            for cn in range(1, NT):
                sl = slice(cn * 512, cn * 512 + 512)
                nc.sync.dma_start(xh[:, :, sl], xr[:, :, sl])

            # b_v broadcast to [128, VD]
            pbv = bc_ps.tile([128, 512], F32, tag="bc")
            nc.tensor.matmul(pbv[:, 0:VD], ones_rb[:],
                             bv_row[:], start=True, stop=True)
            bvbc_sb = xh_pool.tile([128, VD], BF16)
            nc.vector.tensor_copy(bvbc_sb[:], pbv[:, 0:VD])

            inv_c = 1.0 / C
            for P in range(2):
              for cn in range(NT):
                sl = slice(cn * 512, cn * 512 + 512)
                if P == 0:
                    # LN1 stats: sums and sumsqs share one PSUM bank at
                    # partitions 0 and 32 (distinct PE column groups)
                    ps_sq = ln_ps.tile([33, 512], F32, tag="ps")
                    ps_s = ps_sq[0:1, :]
                    ps_q = ps_sq[32:33, :]
                    for o in range(CO):
                        sq = sq_pool.tile([128, 512], BF16)
                        nc.vector.tensor_mul(sq[:], xh[:, o, sl],
                                             xh[:, o, sl])
                        nc.tensor.matmul(ps_s, ones_cb[:], xh[:, o, sl],
                                         start=(o == 0), stop=(o == CO - 1))
                        nc.tensor.matmul(ps_q, ones_cb[:], sq[:],
                                         start=(o == 0), stop=(o == CO - 1))
                    mu = rows.tile([1, 512], F32, tag="mu")
                    msq = rows.tile([1, 512], F32, tag="msq")
                    var = rows.tile([1, 512], F32, tag="var")
                    lnv = rows.tile([1, 512], F32, tag="lnv")
                    rstd = rows.tile([1, 512], F32, tag="rstd")
                    nc.vector.tensor_scalar_mul(mu[:], ps_s, inv_c)
                    nc.vector.tensor_scalar_mul(msq[:], ps_q, inv_c)
                    nc.vector.tensor_mul(var[:], mu[:], mu[:])
                    nc.vector.tensor_tensor(var[:], msq[:], var[:], SUB)
                    # rstd = exp(-0.5*ln(var+EPS)): Ln and Exp live in the
                    # same ACT table set as attention's exp -> no reloads
                    nc.scalar.activation(lnv[:], var[:], AF.Ln, bias=eps_sb[:])
                    nc.scalar.activation(rstd[:], lnv[:], AF.Exp, scale=-0.5)
                    rstd_bf = rows.tile([1, 512], BF16, tag="rstd_bf")
                    nm_bf = rows.tile([1, 512], BF16, tag="nm_bf")
                    nc.vector.tensor_copy(rstd_bf[:], rstd[:])
                    nc.vector.scalar_tensor_tensor(nm_bf[:], mu[:], -1.0,
                                                   rstd[:], MUL, MUL)

                    pb = bc_ps.tile([128, 512], F32, tag="bc")
                    nc.tensor.matmul(pb[:], ones_rb[:], rstd_bf[:],
                                     start=True, stop=True)
                    rstd_bc = rows.tile([128, 512], BF16, tag="rstd_bc")
                    nc.vector.tensor_copy(rstd_bc[:], pb[:])
                    pb2 = bc_ps.tile([128, 512], F32, tag="bc")
                    nc.tensor.matmul(pb2[:], ones_rb[:], nm_bf[:],
                                     start=True, stop=True)
                    nm_bc = rows.tile([128, 512], BF16, tag="nm_bc")
                    nc.vector.tensor_copy(nm_bc[:], pb2[:])

                    for o in range(CO):
                        nc.vector.tensor_mul(xh[:, o, sl], xh[:, o, sl],
                                             rstd_bc[:])
                        nc.vector.tensor_add(xh[:, o, sl], xh[:, o, sl],
                                             nm_bc[:])

                # q, k for head pair P of this chunk (m=P: q, m=P+2: k)
                for m in (P, P + 2):
                    pq_full = mm_ps.tile([128, 2, 512], F32, tag="mm")
                    pq = pq_full[:, 0, :]
                    for o in range(CO):
                        nc.tensor.matmul(pq,
                                         wqk_sb[:, o, m * 128:(m + 1) * 128],
                                         xh[:, o, sl],
                                         start=(o == 0), stop=(o == CO - 1))
                    dest = q_sb[:, P, sl] if m < 2 else k_sb[:, P, sl]
                    nc.scalar.activation(dest, pq, AF.Identity,
                                         bias=bqk_sb[:, m:m + 1])

                # v (token-major) for heads (2P, 2P+1) of this chunk
                vcl = slice(128 * P, 128 * P + 128)
                for tt in range(4 * cn, 4 * cn + 4):
                    tsl = slice(tt * 128, tt * 128 + 128)
                    pv_full = mm_ps.tile([128, 2, 512], F32, tag="mm",
                                         name="pv")
                    pv = pv_full[:, 0, 0:128]
                    for o in range(CO):
                        nc.tensor.matmul(pv, xh[:, o, tsl],
                                         wv_sb[:, o, vcl],
                                         start=(o == 0), stop=(o == CO - 1))
                    vview = v_sb[:, tt, 2 * P:2 * P + 2, :]
                    nc.vector.tensor_tensor(
                        vview[:, :, 0:D],
                        pv.rearrange("p (h e) -> p h e", e=D),
                        bvbc_sb[:, vcl].rearrange("p (h e) -> p h e", e=D),
                        ADD)

                # attention for pair P, query block cn
                pt, qb = P, cn
                qsl = sl
                nkv = 4 * qb + 4
                a2 = a_pool.tile([128, TT, 2, 512], BF16, tag="a")
                for t in range(nkv):
                    sp = mm_ps.tile([128, 2, 512], F32, tag="mm", name="sp")
                    nc.tensor.matmul(
                        sp[:, 0, :],
                        k_sb[0:64, pt, t * 128:(t + 1) * 128],
                        q_sb[0:64, pt, qsl],
                        start=True, stop=True)
                    nc.tensor.matmul(
                        sp[:, 1, :],
                        k_sb[64:128, pt, t * 128:(t + 1) * 128],
                        q_sb[64:128, pt, qsl],
                        start=True, stop=True)
                    diag = t >= 4 * qb
                    if not diag and (t % 4 == 1 or t % 8 == 6):
                        # Schraudolph exp on DVE (both heads at once)
                        it = i_pool.tile([128, 2, 512], I32)
                        nc.vector.tensor_scalar(
                            it[:], sp[:], A_EXP, B_EXP, MUL, ADD)
                        nc.vector.tensor_copy(a2[:, t, :, :],
                                              it[:].bitcast(F32))
                    else:
                        nc.scalar.activation(a2[:, t, :, :], sp[:],
                                             AF.Exp, scale=0.125)
                        if diag:
                            for s in range(2):
                                nc.vector.tensor_mul(
                                    a2[:, t, s, :], a2[:, t, s, :],
                                    mask_sb[:, t - 4 * qb, :])
                for s in range(2):
                    h = 2 * pt + s
                    py = y_ps.tile([D + 1, 512], F32)
                    for t in range(nkv):
                        nc.tensor.matmul(
                            py[:],
                            v_sb[:, t, h, 0:D + 1],
                            a2[:, t, s, :],
                            start=(t == 0), stop=(t == nkv - 1))
                    stg = stage_pool.tile([D + 1, 512], BF16, tag="stg")
                    nc.vector.tensor_copy(stg[:], py[:])
                    stg_last = stg
                    # scatter [65, 512] -> slots (2qb, 2qb+1), rows 65s
                    dst = cc_in[pt][:].rearrange(
                        "(j r) t -> r j t", j=N_CORES)[
                        65 * s:65 * s + 65, 2 * qb:2 * qb + 2, :]
                    nc.sync.dma_start(
                        dst, stg[:].rearrange("r (s2 t) -> r s2 t", s2=2))

        qkv_cm.__exit__(None, None, None)

        # ---- phase 4: y assemble, x2, LN2, MLP -----------------------
        # Group-0 assembly overlaps A2A#1 (it only needs cc_out[0]).
        with tc.tile_pool(name="mlp", bufs=1) as mlp_pool, \
             tc.tile_pool(name="ln2_ps", bufs=1, space="PSUM") as ln2_ps, \
             tc.tile_pool(name="bc2_ps", bufs=2, space="PSUM") as bc2_ps, \
             tc.tile_pool(name="rows2", bufs=1) as rows2, \
             tc.tile_pool(name="sq2", bufs=3) as sq2_pool, \
             tc.tile_pool(name="wp", bufs=2) as wp_pool, \
             tc.tile_pool(name="m_ps", bufs=3, space="PSUM") as m_ps, \
             tc.tile_pool(name="o_sb", bufs=2) as o_sb:

            wfc_sb = mlp_pool.tile([128, CO, FC], BF16)     # 8 MB
            x2 = mlp_pool.tile([128, CO, TCH], F32)
            nc.sync.dma_start(x2[:], x_res.rearrange("(o p) t -> p o t",
                                                     p=128))
            nc.sync.dma_start(wfc_sb[:], w_fc)
            y_sb = mlp_pool.tile([128, CO, TCH], BF16)
            x2bf = mlp_pool.tile([128, CO, TCH], BF16)
            stat_sb = [rows2.tile([1, TCH], F32, name=f"st{i}")
                       for i in range(4)]    # s0, q0, s1, q1

            def do_collective(pt):
                nc.gpsimd.collective_compute(
                    "AllToAll",
                    mybir.AluOpType.bypass,
                    replica_groups=[list(range(N_CORES))],
                    ins=[cc_in[pt].opt()],
                    outs=[cc_out[pt].opt()],
                )

            def assembly_dmas(pt, den_bf):
                # issued from the gpsimd queue: it already serializes on the
                # collective completion waits, so these sit exactly where
                # they become valid -- putting them on the sync queue would
                # block its FIFO (DMA issue + sem relays) on the collective
                src_all = cc_out[pt][:].rearrange(
                    "(bb g s r) t -> g s r bb t", bb=2, g=4, s=2, r=D + 1)
                for g4 in range(4):
                    o = 2 * g4 + pt
                    for s in range(2):
                        src = src_all[g4, s, 0:D, :, :]       # [64,2,256]
                        dst = y_sb[64 * s:64 * s + 64, o, :].rearrange(
                            "d (bb t) -> d bb t", bb=2)
                        nc.gpsimd.dma_start(dst, src)
                den_src = cc_out[pt][:].rearrange(
                    "(bb hh r) t -> hh r bb t", bb=2, hh=8, r=D + 1)[
                    :, D:D + 1, :, :]
                nc.gpsimd.dma_start(
                    den_bf[:].rearrange("hh (u bb t) -> hh u bb t",
                                        u=1, bb=2),
                    den_src)
            den_bfs = [rows2.tile([8, TCH], BF16, name=f"den{g}")
                       for g in range(2)]
            do_collective(0)
            assembly_dmas(0, den_bfs[0])
            do_collective(1)
            assembly_dmas(1, den_bfs[1])

            inv_c = 1.0 / C
            BYP = mybir.AluOpType.bypass
            for pt in range(2):
                den_bf = den_bfs[pt]
                den_f = rows2.tile([8, TCH], F32, tag="den_f")
                rr_f = rows2.tile([8, TCH], F32, tag="rr_f")
                rr_bf = rows2.tile([8, TCH], BF16, tag="rr_bf")
                # fence: fake-read pt1's last attention output (pt=0) /
                # group-0's last x2bf tile (pt=1) so the scheduler cannot
                # hoist this collective-gated chain ahead of live compute
                # on the DVE queue
                marker = (stg_last[0:8, :] if pt == 0
                          else x2bf[0:8, 6, :])
                nc.vector.scalar_tensor_tensor(den_f[:], den_bf[:], 1.0,
                                               marker, MUL, BYP)
                nc.vector.reciprocal_approx_fast(rr_f[:], den_f[:])
                nc.vector.tensor_copy(rr_bf[:], rr_f[:])

                ps2_sq = ln2_ps.tile([33, TCH], F32, tag="sq")
                ps2_s = ps2_sq[0:1, :]
                ps2_q = ps2_sq[32:33, :]
                for i, g4 in enumerate(range(4)):
                    o = 2 * g4 + pt
                    prr = bc2_ps.tile([128, TCH], F32, tag="bc2")
                    nc.tensor.matmul(prr[:],
                                     sel_sb[pt][:, o * 128:(o + 1) * 128],
                                     rr_bf[:], start=True, stop=True)
                    rrbc = o_sb.tile([128, TCH], BF16, tag="rrbc")
                    nc.vector.tensor_copy(rrbc[:], prr[:])
                    yn = o_sb.tile([128, TCH], F32, tag="yn")
                    nc.vector.tensor_mul(yn[:], y_sb[:, o, :], rrbc[:])
                    nc.vector.tensor_add(x2[:, o, :], x2[:, o, :], yn[:])
                    nc.vector.tensor_copy(x2bf[:, o, :], x2[:, o, :])
                    sq = sq2_pool.tile([128, TCH], BF16)
                    nc.vector.tensor_mul(sq[:], x2bf[:, o, :], x2bf[:, o, :])
                    nc.tensor.matmul(ps2_s, ones_cb[:], x2bf[:, o, :],
                                     start=(i == 0), stop=(i == 3))
                    nc.tensor.matmul(ps2_q, ones_cb[:], sq[:],
                                     start=(i == 0), stop=(i == 3))
                # move group stats to SBUF so the PSUM bank can be reused
                nc.vector.tensor_copy(stat_sb[2 * pt][:], ps2_s)
                nc.vector.tensor_copy(stat_sb[2 * pt + 1][:], ps2_q)

            # LN2 over the 512 on-core tokens
            mu2 = rows2.tile([1, TCH], F32, tag="r_mu2")
            msq2 = rows2.tile([1, TCH], F32, tag="r_msq2")
            var2 = rows2.tile([1, TCH], F32, tag="r_var2")
            nc.vector.tensor_tensor(mu2[:], stat_sb[0][:], stat_sb[2][:], ADD)
            nc.vector.tensor_tensor(msq2[:], stat_sb[1][:], stat_sb[3][:],
                                    ADD)
            nc.vector.tensor_scalar_mul(mu2[:], mu2[:], inv_c)
            nc.vector.tensor_scalar_mul(msq2[:], msq2[:], inv_c)
            nc.vector.tensor_mul(var2[:], mu2[:], mu2[:])
            nc.vector.tensor_tensor(var2[:], msq2[:], var2[:], SUB)
            lnv2 = rows2.tile([1, TCH], F32, tag="r_msq2")   # reuse dead buf
            nc.scalar.activation(lnv2[:], var2[:], AF.Ln, bias=eps_sb[:])
            rstd2 = rows2.tile([1, TCH], F32, tag="r_var2")  # var2 dead too
            nc.scalar.activation(rstd2[:], lnv2[:], AF.Exp, scale=-0.5)
            rstd2_bf = rows2.tile([1, TCH], BF16)
            nm2_bf = rows2.tile([1, TCH], BF16)
            nc.vector.tensor_copy(rstd2_bf[:], rstd2[:])
            nc.vector.scalar_tensor_tensor(nm2_bf[:], mu2[:], -1.0, rstd2[:],
                                           MUL, MUL)

            pb = bc2_ps.tile([128, TCH], F32, tag="bc2")
            nc.tensor.matmul(pb[:], ones_rb[:], rstd2_bf[:],
                             start=True, stop=True)
            rstd2_bc = rows2.tile([128, TCH], BF16)
            nc.vector.tensor_copy(rstd2_bc[:], pb[:])
            pb2 = bc2_ps.tile([128, TCH], F32, tag="bc2")
            nc.tensor.matmul(pb2[:], ones_rb[:], nm2_bf[:],
                             start=True, stop=True)
            nm2_bc = rows2.tile([128, TCH], BF16)
            nc.vector.tensor_copy(nm2_bc[:], pb2[:])

            # fc + gelu on RAW x2bf; the LN2 normalization is deferred into
            # a per-token scale/shift on the (otherwise idle) vector engine:
            #   fc(LN(x)) = rstd_t * fc_raw(x)_mt + (-mu*rstd)_t * S_m + b_m
            m_sb = mlp_pool.tile([128, FO, TCH], BF16)
            for mt in range(FO):
                pm = m_ps.tile([128, TCH], F32, tag="mm2")
                for o in range(CO):
                    nc.tensor.matmul(pm[:],
                                     wfc_sb[:, o, mt * 128:(mt + 1) * 128],
                                     x2bf[:, o, :],
                                     start=(o == 0), stop=(o == CO - 1))
                shift = o_sb.tile([128, TCH], BF16, tag="shift")
                nc.scalar.activation(shift[:], nm2_bc[:], AF.Identity,
                                     bias=bfc_sb[:, mt:mt + 1],
                                     scale=sfc_sb[:, mt:mt + 1])
                tmp = o_sb.tile([128, TCH], BF16, tag="fcraw")
                nc.vector.tensor_mul(tmp[:], pm[:], rstd2_bc[:])
                nc.vector.tensor_add(tmp[:], tmp[:], shift[:])
                nc.scalar.activation(m_sb[:, mt, :], tmp[:], AF.Gelu)

            # proj + bias + residual (weights streamed per o-tile)
            out_r = out_t.rearrange("(o p) t -> p o t", p=128)
            wpj_r = w_pj.rearrange("(o p) (k m) -> o p k m", p=128, m=128)
            for o in range(CO):
                wt = wp_pool.tile([128, FO, 128], BF16, tag="wpj")
                nc.sync.dma_start(wt[:], wpj_r[o])
                pp = m_ps.tile([128, TCH], F32, tag="mm2")
                for kt in range(FO):
                    nc.tensor.matmul(pp[:], wt[:, kt, :], m_sb[:, kt, :],
                                     start=(kt == 0), stop=(kt == FO - 1))
                po_sb = o_sb.tile([128, TCH], F32, tag="po")
                nc.scalar.activation(po_sb[:], pp[:], AF.Identity,
                                     bias=bpj_sb[:, o:o + 1])
                fin = o_sb.tile([128, TCH], F32, tag="fin")
                nc.vector.tensor_add(fin[:], po_sb[:], x2[:, o, :])
                nc.sync.dma_start(out_r[:, o, :], fin[:])

        for cm in (dram_cm, const_cm):
            cm.__exit__(None, None, None)

    nc.compile()
    return nc


def _get_nc():
    if "nc" not in _CACHE:
        _CACHE["nc"] = _build()
    return _CACHE["nc"]


def _make_masks():
    m = np.zeros((4, 128, 512), np.float32)
    i = np.arange(128)[:, None]
    j = np.arange(512)[None, :]
    for t in range(4):
        m[t] = (128 * t + i <= j).astype(np.float32)
    return m


def kernel(x, ln1_g, ln1_b, W_attn, b_attn, ln2_g, ln2_b, W_fc, b_fc,
           W_proj, b_proj):
    global LAST_EXEC_NS, LAST_RESULTS
    import os
    import ml_dtypes

    from concourse.bass_utils import run_bass_kernel_spmd

    BF = ml_dtypes.bfloat16

    x = np.asarray(x, np.float32)
    W1 = np.asarray(ln1_g, np.float32)[:, None] * np.asarray(W_attn, np.float32)
    b1 = np.asarray(b_attn, np.float32) + \
        np.asarray(ln1_b, np.float32) @ np.asarray(W_attn, np.float32)
    Wf = np.asarray(ln2_g, np.float32)[:, None] * np.asarray(W_fc, np.float32)
    bf = np.asarray(b_fc, np.float32) + \
        np.asarray(ln2_b, np.float32) @ np.asarray(W_fc, np.float32)
    Wp = np.asarray(W_proj, np.float32)
    bp = np.asarray(b_proj, np.float32)

    masks = _make_masks().astype(BF)

    wfc_l = np.ascontiguousarray(
        Wf.reshape(CO, 128, FC).transpose(1, 0, 2)).astype(BF)
    # wt[p, kt, m] must equal Wp[kt*128+p, o*128+m]
    wpj_l = np.ascontiguousarray(
        Wp.reshape(FO, 128, CO, 128).transpose(2, 1, 0, 3).reshape(C, FC)
    ).astype(BF)
    bfc_l = np.ascontiguousarray(bf.reshape(FO, 128).T)
    bpj_l = np.ascontiguousarray(bp.reshape(CO, 128).T)
    sfc_l = np.ascontiguousarray(
        Wf.sum(axis=0).astype(np.float32).reshape(FO, 128).T)

    # per-group one-hot: rrbc[p, t] = rr[2*g4 + p//64, t] for o = 2*g4+pt
    sels = []
    for pt in range(2):
        sg = np.zeros((8, C), np.float32)
        for g4 in range(4):
            o = 2 * g4 + pt
            for s in range(2):
                sg[2 * g4 + s, o * 128 + 64 * s:o * 128 + 64 * s + 64] = 1.0
        sels.append(sg.astype(BF))

    xT = [np.ascontiguousarray(x[b].T) for b in range(B)]

    in_maps = []
    for c in range(N_CORES):
        b = c // 4
        g = c % 4
        qc = slice(g * HPC * D, (g + 1) * HPC * D)
        kc = slice(C + g * HPC * D, C + (g + 1) * HPC * D)
        vc = slice(2 * C + g * HPC * D, 2 * C + (g + 1) * HPC * D)
        wqk = np.concatenate([W1[:, qc], W1[:, kc]], axis=1)      # [1024,512]
        wv = W1[:, vc]                                            # [1024,256]
        tok0 = TB * c
        xres = np.ascontiguousarray(np.concatenate(
            [xT[0][:, tok0:tok0 + TB], xT[1][:, tok0:tok0 + TB]], axis=1))
        in_maps.append({
            "x_t": xT[b].astype(BF),
            "x_res": xres,
            "w_qk": np.ascontiguousarray(
                wqk.reshape(CO, 128, 512).transpose(1, 0, 2)).astype(BF),
            "b_qk": np.ascontiguousarray(
                np.concatenate([b1[qc], b1[kc]]).reshape(4, 128).T),
            "w_v": np.ascontiguousarray(
                wv.reshape(CO, 128, VD).transpose(1, 0, 2)).astype(BF),
            "b_v": np.ascontiguousarray(b1[vc][None, :]).astype(BF),
            "masks": masks,
            "w_fc": wfc_l,
            "b_fc": bfc_l,
            "w_pj": wpj_l,
            "b_pj": bpj_l,
            "s_fc": sfc_l,
            "sel0": sels[0],
            "sel1": sels[1],
            "ones_col": np.ones((128, 1), np.float32).astype(BF),
            "ones_row": np.ones((1, 128), np.float32).astype(BF),
            "v_ones": np.ones((128, TT, HPC, 1), np.float32).astype(BF),
        })

    nc = _get_nc()
    trace = os.environ.get("KERNEL_TRACE") == "1"
    kw = {}
    if trace:
        kw = dict(trace=True, trace_cores=list(range(N_CORES)))
    res = run_bass_kernel_spmd(nc, in_maps, core_ids=list(range(N_CORES)), **kw)
    LAST_EXEC_NS = res.exec_time_ns
    LAST_RESULTS = res

    out = np.empty((B, T, C), np.float32)
    for c in range(N_CORES):
        tok0 = TB * c
        r = res.results[c]["out_t"]
        out[0, tok0:tok0 + TB, :] = r[:, 0:TB].T
        out[1, tok0:tok0 + TB, :] = r[:, TB:2 * TB].T
    return out
